# revision 58
# baseline (speedup 1.0000x reference)
"""Trainium2 Bass kernel for the gammatone-cochlea + LIF-SNN model (v2).

Numerically the SNN is chaotic: input perturbations of ~1e-5 relative flip
hundreds of output spikes (measured), so every value feeding a threshold
comparison must match the fp32 reference to <1e-6.  fp32 PE matmuls cost 4
cycles/col in the cost model; float32r costs 1 but its hardware numerics
(bf16 triple-product, ~3.4e-5) fail the gate.  Instead the conv runs as an
exact fp16 split: audio a = ah+al and kernel g = gh+gl (each an exact fp16
pair), and y = ah*gh + ah*gl + al*gh accumulated in fp32 PSUM (the dropped
al*gl term is ~2^-24, measured zero spike flips end-to-end).  Six 1-cyc/col
passes instead of two 4-cyc/col passes: 25% faster PE and exact.

Pipeline per core (32 of the 256 batch rows, pure data parallel), emitted
strip-major so the SNN wavefronts under the conv (PE ~92% busy end to end):
  for si in strips(2048 samples = 16 frames):
    for g in groups(4 rows): Hankel DMA (fp16 hi+lo) -> 6 matmul passes per
      512-block -> ReLU (Act, psum->sbuf) -> 128-block sums (DVE reduce)
    twice per strip (between conv groups, so the in-order PE queue never
    waits on LIF progress): env chunk (DVE) -> selector matmuls (PE, fp32)
      -> AN spikes (DVE, fp16 0/1) -> bushy currents (PE, fp16-split
      weights, exact on 0/1 spikes) -> Act copy to SBUF -> LIF chains
      (bushy chunk k, ic chunk k-1, ac chunk k-2, all on DVE: STT
      u=beta*mem+cur, is_gt spike, subtract reset; 3 ops/step/layer)
The last strip's chunks are split 7/4/3 frames and drained chunk-pipelined
after the conv using a 2-op negated-state form (u = -beta*m' + cur,
m' = (u>1) - u, bit-identical to the 3-op form) with spikes/outputs derived
in bulk per chunk, shortening the serial post-conv cascade; finished frames
stream out over a split output DMA.
Outputs [10, 124*32] (t-major) per core; host reassembles to [B, T, 10].
"""
import numpy as np
import concourse.bass as bass
import concourse.bacc as bacc
import concourse.mybir as mybir
import concourse.tile as tile
from concourse.bass_utils import run_bass_kernel_spmd

dt = mybir.dt
AF = mybir.ActivationFunctionType
OP = mybir.AluOpType

NCORES = 8
B, N, C, K = 256, 16000, 32, 64
BLOC = B // NCORES            # 32 batch rows per core
WINDOW, STRIDE, T = 256, 128, 124
ANS, HID, OUT = 10, 50, 10
BETA, THR, AN_THR = 0.95, 1.0, 0.5
PAD_L, PAD_R = 31, 33         # SAME padding for K=64: 31 left, 32 right (+1 slack)
NPAD = PAD_L + N + PAD_R      # 16064
FREE = T * BLOC               # 3968
NGRP = BLOC // 4              # 8 groups of 4 rows
STRIPS = [2048] * 7 + [1664]
NSTRIP = len(STRIPS)
S0S = [sum(STRIPS[:i]) for i in range(NSTRIP)]
# frame chunks: wave w = chunks ready once strip w's block sums exist.
# 7-8 frame chunks keep the ic/ac wavefront lag (and the post-conv drain)
# short.
CHUNKS = [(0, 7), (7, 14)] + \
    [c for i in range(1, 7)
     for c in ((16 * i - 2, 16 * i + 6), (16 * i + 6, 16 * i + 14))] + \
    [(110, 117), (117, 121), (121, 124)]

# jnp.linspace(0.5, 1.5, 10, dtype=f32), bitexact
_SCALES = np.array([0x3F000000, 0x3F1C71C7, 0x3F38E38E, 0x3F555555, 0x3F71C71D,
                    0x3F871C72, 0x3F955556, 0x3FA38E39, 0x3FB1C71D, 0x3FC00000],
                   dtype=np.uint32).view(np.float32)

_NC_CACHE = None


def _build_nc():
    nc = bacc.Bacc("TRN2", target_bir_lowering=False, debug=False,
                   num_devices=NCORES)

    aph = nc.dram_tensor("aph", [BLOC, NPAD], dt.float16, kind="ExternalInput")
    apl = nc.dram_tensor("apl", [BLOC, NPAD], dt.float16, kind="ExternalInput")
    # gammatone lhsT blocks: [hi/lo][tap-half] block-diagonal [128, 128]
    lg = nc.dram_tensor("lg", [4, 128, 128], dt.float16, kind="ExternalInput")
    wb = nc.dram_tensor("wb", [6, 128, HID], dt.float16, kind="ExternalInput")
    wic = nc.dram_tensor("wic", [2, HID, HID], dt.float16, kind="ExternalInput")
    wac = nc.dram_tensor("wac", [2, HID, OUT], dt.float16, kind="ExternalInput")
    sv = nc.dram_tensor("sv", [128, 3], dt.float32, kind="ExternalInput")
    selr = nc.dram_tensor("selr", [4, 128, 128], dt.float32, kind="ExternalInput")
    ospk = nc.dram_tensor("ospk", [OUT, FREE], dt.float32, kind="ExternalOutput")
    omem = nc.dram_tensor("omem", [OUT, FREE], dt.float32, kind="ExternalOutput")

    with tile.TileContext(nc) as tc:
        with tc.tile_pool(name="cpool", bufs=1) as cp:
            lgt = [cp.tile([128, 128], dt.float16, name=f"lgt{i}")
                   for i in range(4)]
            for i in range(4):
                nc.scalar.dma_start(out=lgt[i][:, :], in_=lg[i, :, :])
            svt = cp.tile([128, 3], dt.float32)
            nc.gpsimd.dma_start(out=svt[:, :], in_=sv[:, :])
            wbt = [cp.tile([128, HID], dt.float16, name=f"wbt{i}")
                   for i in range(6)]
            for i in range(6):
                nc.gpsimd.dma_start(out=wbt[i][:, :], in_=wb[i, :, :])
            wict = [cp.tile([HID, HID], dt.float16, name=f"wict{i}")
                    for i in range(2)]
            wact = [cp.tile([HID, OUT], dt.float16, name=f"wact{i}")
                    for i in range(2)]
            for i in range(2):
                nc.gpsimd.dma_start(out=wict[i][:, :], in_=wic[i, :, :])
                nc.gpsimd.dma_start(out=wact[i][:, :], in_=wac[i, :, :])
            selt = [cp.tile([128, 128], dt.float32, name=f"selt{r}")
                    for r in range(4)]
            for r in range(4):
                nc.gpsimd.dma_start(out=selt[r][:, :], in_=selr[r, :, :])

            S_all = cp.tile([128, NGRP * 126], dt.float32)
            env_all = cp.tile([128, NGRP * T], dt.float32)
            ospk_t = cp.tile([OUT, FREE], dt.float32,
                             padded_shape=[OUT, FREE + 32])
            omem_t = cp.tile([OUT, FREE], dt.float32,
                             padded_shape=[OUT, FREE + 32])

            # persistent LIF state
            memb = cp.tile([HID, BLOC], dt.float32)
            memic = cp.tile([HID, BLOC], dt.float32)
            zac = cp.tile([OUT, BLOC], dt.float32)
            nc.vector.memset(memb[:, :], 0.0)
            nc.vector.memset(memic[:, :], 0.0)
            nc.gpsimd.memset(zac[:, :], 0.0)

            hkp = tc.alloc_tile_pool(name="hkp", bufs=8)
            ysp = tc.alloc_tile_pool(name="ysp", bufs=4)
            anp = tc.alloc_tile_pool(name="anp", bufs=3)
            sbp = tc.alloc_tile_pool(name="sbp", bufs=1)
            psc = tc.alloc_tile_pool(name="psc", bufs=3, space="PSUM")
            pss = tc.alloc_tile_pool(name="pss", bufs=2, space="PSUM")
            psl = tc.alloc_tile_pool(name="psl", bufs=3, space="PSUM")

            def conv_strip(g, si):
                sw = STRIPS[si]
                s0 = S0S[si]
                hkh = hkp.tile([128, 2080], dt.float16, tag="hk", name="hkh")
                hkl = hkp.tile([128, 2080], dt.float16, tag="hk", name="hkl")
                # hk[32r+k, j] = ap[4g+r, s0+j+k] : one DMA, 128 partition rows
                if si == 0 and g == 0:
                    # split the very first transfers so the first conv matmul
                    # can start after only half a strip has landed
                    for (t, ap_) in ((hkh, aph), (hkl, apl)):
                        for (a, b) in ((0, 1056), (1056, sw + 32)):
                            src = bass.AP(ap_, 4 * g * NPAD + s0 + a,
                                          [[NPAD, 4], [1, 32], [1, b - a]])
                            nc.sync.dma_start(out=t[:, a:b], in_=src)
                else:
                    srch = bass.AP(aph, 4 * g * NPAD + s0,
                                   [[NPAD, 4], [1, 32], [1, sw + 32]])
                    srcl = bass.AP(apl, 4 * g * NPAD + s0,
                                   [[NPAD, 4], [1, 32], [1, sw + 32]])
                    nc.sync.dma_start(out=hkh[:, 0:sw + 32], in_=srch)
                    nc.sync.dma_start(out=hkl[:, 0:sw + 32], in_=srcl)
                ys = ysp.tile([128, 2048], dt.float32, tag="ys", name="ys")
                nb4 = (sw + 511) // 512
                for b4 in range(nb4):
                    w = min(512, sw - 512 * b4)
                    o = 512 * b4
                    acc = psc.tile([128, 512], dt.float32, tag="acc",
                                   name="acc")
                    # y = ah*gh + ah*gl + al*gh  (two tap-halves each)
                    nc.tensor.matmul(acc[:, 0:w], lgt[0][:, :],
                                     hkh[:, o:o + w], start=True, stop=False)
                    nc.tensor.matmul(acc[:, 0:w], lgt[1][:, :],
                                     hkh[:, o + 32:o + 32 + w],
                                     start=False, stop=False)
                    nc.tensor.matmul(acc[:, 0:w], lgt[2][:, :],
                                     hkh[:, o:o + w], start=False, stop=False)
                    nc.tensor.matmul(acc[:, 0:w], lgt[3][:, :],
                                     hkh[:, o + 32:o + 32 + w],
                                     start=False, stop=False)
                    nc.tensor.matmul(acc[:, 0:w], lgt[0][:, :],
                                     hkl[:, o:o + w], start=False, stop=False)
                    nc.tensor.matmul(acc[:, 0:w], lgt[1][:, :],
                                     hkl[:, o + 32:o + 32 + w],
                                     start=False, stop=True)
                    nc.scalar.activation(ys[:, o:o + w], acc[:, 0:w], AF.Relu)
                nblk = sw // 128
                b0 = s0 // 128
                if si == NSTRIP - 1:
                    # split the last strip's block sums: the first 8 blocks
                    # gate the drain's first LIF chunk, so finishing them
                    # before the tail blocks starts the drain sooner
                    for (ba, bb) in ((0, 8), (8, nblk)):
                        view = bass.AP(ys.tensor, ys.offset + 128 * ba,
                                       [list(ys.ap[0]), [128, bb - ba],
                                        [1, 128]])
                        nc.vector.tensor_reduce(
                            S_all[:, g * 126 + b0 + ba:
                                  g * 126 + b0 + bb],
                            view, axis=mybir.AxisListType.X, op=OP.add)
                else:
                    view = bass.AP(ys.tensor, ys.offset,
                                   [list(ys.ap[0]), [128, nblk], [1, 128]])
                    nc.vector.tensor_reduce(
                        S_all[:, g * 126 + b0: g * 126 + b0 + nblk],
                        view, axis=mybir.AxisListType.X, op=OP.add)

            def an_chunk(ci):
                t0, t1 = CHUNKS[ci]
                w = t1 - t0
                # env256[t] = S[t] + S[t+1]  (the /256 is folded into sv)
                dst = bass.AP(env_all.tensor, env_all.offset + t0,
                              [list(env_all.ap[0]), [T, NGRP], [1, w]])
                sA = bass.AP(S_all.tensor, S_all.offset + t0,
                             [list(S_all.ap[0]), [126, NGRP], [1, w]])
                sB = bass.AP(S_all.tensor, S_all.offset + t0 + 1,
                             [list(S_all.ap[0]), [126, NGRP], [1, w]])
                nc.vector.tensor_tensor(dst, sA, sB, OP.add)
                # replicate to (scale-block u, channel) partitions via 0/1
                # selector matmuls; chunk cols are b*w + dt, b = 4g+r
                shf = pss.tile([128, 32 * w], dt.float32, tag="shf",
                               name="shf")
                for r in range(4):
                    outap = bass.AP(shf.tensor, shf.offset + r * w,
                                    [list(shf.ap[0]), [4 * w, NGRP], [1, w]])
                    rhs = bass.AP(env_all.tensor, env_all.offset + t0,
                                  [list(env_all.ap[0]), [T, NGRP], [1, w]])
                    nc.tensor.matmul(outap, selt[r][:, :], rhs,
                                     start=True, stop=True)
                # AN spikes (3 passes cover the 10 scales) + bushy currents
                pb = psl.tile([HID, 32 * w], dt.float32, tag="pcur",
                              name="pb")
                for ch in range(3):
                    an = anp.tile([128, 32 * w], dt.float16, tag="an",
                                  name="an")
                    nc.vector.tensor_scalar(an[:, :], shf[:, :],
                                            svt[:, ch:ch + 1], AN_THR,
                                            OP.mult, OP.is_gt)
                    nc.tensor.matmul(pb[:, :], wbt[ch][:, :], an[:, :],
                                     start=(ch == 0), stop=False)
                    nc.tensor.matmul(pb[:, :], wbt[3 + ch][:, :], an[:, :],
                                     start=False, stop=(ch == 2))
                cb = sbp.tile([HID, 32 * w], dt.float32, tag="cb", bufs=2,
                              name="cb")
                nc.scalar.activation(cb[:, :], pb[:, :], AF.Copy)
                return cb

            def ic_cur(ci, sb):
                t0, t1 = CHUNKS[ci]
                w = t1 - t0
                pi = psl.tile([HID, 32 * w], dt.float32, tag="pcur",
                              name="pi")
                nc.tensor.matmul(pi[:, :], wict[0][:, :], sb[:, :],
                                 start=True, stop=False)
                nc.tensor.matmul(pi[:, :], wict[1][:, :], sb[:, :],
                                 start=False, stop=True)
                cic = sbp.tile([HID, 32 * w], dt.float32, tag="cic", bufs=2,
                               name="cic")
                nc.scalar.activation(cic[:, :], pi[:, :], AF.Copy)
                return cic

            def ac_cur(ci, sic):
                t0, t1 = CHUNKS[ci]
                w = t1 - t0
                pa = psl.tile([OUT, 32 * w], dt.float32, tag="pcur",
                              name="pa")
                nc.tensor.matmul(pa[:, :], wact[0][:, :], sic[:, :],
                                 start=True, stop=False)
                nc.tensor.matmul(pa[:, :], wact[1][:, :], sic[:, :],
                                 start=False, stop=True)
                cac = sbp.tile([OUT, 32 * w], dt.float32, tag="cac", bufs=2,
                               name="cac")
                nc.scalar.activation(cac[:, :], pa[:, :], AF.Copy)
                return cac

            def csl(tile2d, w, dtc):
                """column slice {b*w + dtc} of a chunk tile."""
                return bass.AP(tile2d.tensor, tile2d.offset + dtc,
                               [list(tile2d.ap[0]), [w, BLOC]])

            def bushy_steps(ci, cb, ub, sbch):
                t0, t1 = CHUNKS[ci]
                w = t1 - t0
                for dtc in range(w):
                    nc.vector.scalar_tensor_tensor(
                        ub[:, :], memb[:, :], BETA, csl(cb, w, dtc),
                        OP.mult, OP.add)
                    yield
                    nc.vector.tensor_scalar(csl(sbch, w, dtc), ub[:, :],
                                            THR, None, OP.is_gt)
                    yield
                    nc.vector.tensor_tensor(memb[:, :], ub[:, :],
                                            csl(sbch, w, dtc), OP.subtract)
                    yield

            def ic_steps(ci, cic, uic, sicch):
                t0, t1 = CHUNKS[ci]
                w = t1 - t0
                for dtc in range(w):
                    nc.vector.scalar_tensor_tensor(
                        uic[:, :], memic[:, :], BETA, csl(cic, w, dtc),
                        OP.mult, OP.add)
                    yield
                    nc.vector.tensor_scalar(csl(sicch, w, dtc), uic[:, :],
                                            THR, None, OP.is_gt)
                    yield
                    nc.vector.tensor_tensor(memic[:, :], uic[:, :],
                                            csl(sicch, w, dtc), OP.subtract)
                    yield

            def ac_steps(ci, cac, uac):
                # ospk_t/omem_t are t-major: frame t = cols [32t, 32t+32)
                t0, t1 = CHUNKS[ci]
                w = t1 - t0
                for dtc in range(w):
                    t = t0 + dtc
                    prev = (zac[:, :] if t == 0
                            else omem_t[:, 32 * t - 32:32 * t])
                    nc.vector.scalar_tensor_tensor(
                        uac[:, :], prev, BETA, csl(cac, w, dtc),
                        OP.mult, OP.add)
                    yield
                    nc.vector.tensor_scalar(ospk_t[:, 32 * t:32 * t + 32],
                                            uac[:, :], THR, None, OP.is_gt)
                    yield
                    nc.vector.tensor_tensor(omem_t[:, 32 * t:32 * t + 32],
                                            uac[:, :],
                                            ospk_t[:, 32 * t:32 * t + 32],
                                            OP.subtract)
                    yield

            def run_chains(chains):
                its = [c for c in chains if c is not None]
                done = [False] * len(its)
                while not all(done):
                    for k, it in enumerate(its):
                        if not done[k]:
                            try:
                                next(it)
                            except StopIteration:
                                done[k] = True

            # ---------------- main wavefront ----------------
            sb_ch = {}      # bushy spike chunks (fp16)
            sic_ch = {}     # ic spike chunks

            def bushy_for(ci):
                cb = an_chunk(ci)
                wd = CHUNKS[ci][1] - CHUNKS[ci][0]
                ub = sbp.tile([HID, BLOC], dt.float32, tag="ub",
                              bufs=2, name="ub")
                sbch = sbp.tile([HID, 32 * wd], dt.float16,
                                tag="sbch", bufs=4, name="sbch")
                sb_ch[ci] = sbch
                yield from bushy_steps(ci, cb, ub, sbch)

            def ic_for(ci):
                cic = ic_cur(ci, sb_ch.pop(ci))
                wd = CHUNKS[ci][1] - CHUNKS[ci][0]
                uic = sbp.tile([HID, BLOC], dt.float32,
                               tag="uic", bufs=2, name="uic")
                sicch = sbp.tile([HID, 32 * wd], dt.float16,
                                 tag="sicch", bufs=4, name="sicch")
                sic_ch[ci] = sicch
                yield from ic_steps(ci, cic, uic, sicch)

            def ac_for(ci):
                cac = ac_cur(ci, sic_ch.pop(ci))
                uac = sbp.tile([OUT, BLOC], dt.float32,
                               tag="uac", bufs=2, name="uac")
                yield from ac_steps(ci, cac, uac)

            def drain_layer(cis, P, get_cur, on_chunk, state0, tg, ac_u=None):
                """Drain-only 2-op LIF: u = (state*±beta)+cur, m' = (u>1)-u.
                The state is the previous m' column (negated mem) except the
                very first step, which reads the in-loop positive state with
                +beta (exact: -beta*m' == beta*mem bit-for-bit).  For the ac
                layer, u goes t-major into ac_u so spk/mem derive in bulk."""
                prev, pb = state0, BETA
                for ci in cis:
                    t0, t1 = CHUNKS[ci]
                    w = t1 - t0
                    cur = get_cur(ci)
                    mch = sbp.tile([P, 32 * w], dt.float32, tag="dm" + tg,
                                   bufs=3, name="m2" + tg)
                    if ac_u is None:
                        uch = sbp.tile([P, 32 * w], dt.float32, tag="du" + tg,
                                       bufs=3, name="u2" + tg)
                        def ucol(d, _u=uch, _w=w):
                            return csl(_u, _w, d)
                    else:
                        def ucol(d, _t0=t0):
                            a = 32 * (_t0 - 94 + d)
                            return ac_u[:, a:a + 32]
                    for dtc in range(w):
                        nc.vector.scalar_tensor_tensor(
                            ucol(dtc), prev, pb, csl(cur, w, dtc),
                            OP.mult, OP.add)
                        yield
                        u = ucol(dtc)
                        nc.vector.scalar_tensor_tensor(
                            csl(mch, w, dtc), u, THR, u,
                            OP.is_gt, OP.subtract)
                        yield
                        prev, pb = csl(mch, w, dtc), -BETA
                    if ac_u is None:
                        on_chunk(ci, uch, w)

            def drain_bushy_done(ci, uch, w):
                sbch = sbp.tile([HID, 32 * w], dt.float16, tag="sbch",
                                bufs=4, name="sbch")
                nc.vector.tensor_scalar(sbch[:, :], uch[:, :], THR, None,
                                        OP.is_gt)
                sb_ch[ci] = sbch

            def drain_ic_done(ci, uch, w):
                sicch = sbp.tile([HID, 32 * w], dt.float16, tag="sicch",
                                 bufs=4, name="sicch")
                nc.vector.tensor_scalar(sicch[:, :], uch[:, :], THR, None,
                                        OP.is_gt)
                sic_ch[ci] = sicch

            def seq(factory, cis):
                for ci in cis:
                    yield from factory(ci)

            def chunk_work(b_ci, ic_ci, ac_ci):
                """One chunk per chain, staggered: every matmul depends only
                on work finished before this point, so the in-order PE queue
                never stalls on LIF progress."""
                nch = len(CHUNKS)
                chains = []
                if b_ci is not None and 0 <= b_ci < nch:
                    chains.append(bushy_for(b_ci))
                if ic_ci is not None and 0 <= ic_ci < nch:
                    chains.append(ic_for(ic_ci))
                if ac_ci is not None and 0 <= ac_ci < nch:
                    chains.append(ac_for(ac_ci))
                run_chains(chains)

            # PE warmup: ~3us of tiny serialized matmuls while the first
            # Hankel DMA lands, so the real conv starts at full clock (the
            # cost model ramps the PE to peak after 3us of continuous busy)
            wz = cp.tile([128, 64], dt.float16)
            nc.vector.memset(wz[:, :], 0.0)
            pw = psc.tile([128, 512], dt.float32, tag="acc", name="warm")
            for _ in range(34):
                nc.tensor.matmul(pw[0:64, 0:64], wz[:, :], wz[:, :],
                                 start=True, stop=True)

            for si in range(NSTRIP):
                conv_strip(0, si)
                if si >= 1:
                    # chunk work for the previous strip, emitted between
                    # conv groups so the env/spike chains are ready by the
                    # time the PE reaches their matmuls
                    chunk_work(2 * si - 2, 2 * si - 3, 2 * si - 4)
                for g in range(1, 4):
                    conv_strip(g, si)
                if si >= 1:
                    chunk_work(2 * si - 1, 2 * si - 2, 2 * si - 3)
                for g in range(4, NGRP):
                    conv_strip(g, si)
            # frames < 94 are final once ac chunk 11 is done (in-loop);
            # ship them while the drain runs
            CUT = 32 * 94
            nc.sync.dma_start(out=ospk[:, 0:CUT], in_=ospk_t[:, 0:CUT])
            nc.sync.dma_start(out=omem[:, 0:CUT], in_=omem_t[:, 0:CUT])
            # post-conv drain: chunk-pipelined 2-op chains; the ac membrane
            # u is collected t-major in uacd and spk/mem derived in bulk
            uacd = cp.tile([OUT, 32 * 30], dt.float32)
            run_chains([
                drain_layer([14, 15, 16], HID,
                            lambda ci: an_chunk(ci), drain_bushy_done,
                            memb[:, :], "b"),
                drain_layer([13, 14, 15, 16], HID,
                            lambda ci: ic_cur(ci, sb_ch.pop(ci)),
                            drain_ic_done, memic[:, :], "i"),
                drain_layer([12, 13, 14, 15, 16], OUT,
                            lambda ci: ac_cur(ci, sic_ch.pop(ci)),
                            None, omem_t[:, 32 * 93:32 * 94], "a",
                            ac_u=uacd),
            ])
            dsl = slice(32 * 94, FREE)
            nc.vector.tensor_scalar(ospk_t[:, dsl], uacd[:, :], THR, None,
                                    OP.is_gt)
            nc.vector.tensor_tensor(omem_t[:, dsl], uacd[:, :],
                                    ospk_t[:, dsl], OP.subtract)
            nc.sync.dma_start(out=ospk[:, CUT:FREE], in_=ospk_t[:, CUT:FREE])
            nc.sync.dma_start(out=omem[:, CUT:FREE], in_=omem_t[:, CUT:FREE])

            nc.sync.dma_start(out=ospk[:, CUT:FREE], in_=ospk_t[:, CUT:FREE])
            nc.sync.dma_start(out=omem[:, CUT:FREE], in_=omem_t[:, CUT:FREE])

            psl.release()
            pss.release()
            psc.release()
            sbp.release()
            anp.release()
            ysp.release()
            hkp.release()

    nc.finalize()
    return nc


def _f16_split(x):
    hi = x.astype(np.float16)
    lo = (x.astype(np.float32) - hi.astype(np.float32)).astype(np.float16)
    return hi, lo


def _prep_inputs(audio, gt_kernels, W_bushy, W_ic, W_ac):
    audio = np.ascontiguousarray(audio, dtype=np.float32)
    gt = np.ascontiguousarray(gt_kernels, dtype=np.float32)
    Wb = np.ascontiguousarray(W_bushy, dtype=np.float32)

    gh, gl = _f16_split(gt)
    lg = np.zeros((4, 128, 128), np.float16)
    for r in range(4):
        # lhsT[r*32+k, r*32+c] = g[c, k]; [0]=gh taps0-31, [1]=gh taps32-63,
        # [2]=gl taps0-31, [3]=gl taps32-63
        lg[0, r * 32:r * 32 + 32, r * 32:r * 32 + 32] = gh[:, 0:32].T
        lg[1, r * 32:r * 32 + 32, r * 32:r * 32 + 32] = gh[:, 32:64].T
        lg[2, r * 32:r * 32 + 32, r * 32:r * 32 + 32] = gl[:, 0:32].T
        lg[3, r * 32:r * 32 + 32, r * 32:r * 32 + 32] = gl[:, 32:64].T

    wbh, wbl = _f16_split(Wb)       # [50, 320]
    wb = np.zeros((6, 128, HID), np.float16)
    sv = np.zeros((128, 3), np.float32)
    for ch in range(3):
        for u in range(4):
            a = ch * 4 + u
            if a >= ANS:
                continue
            wb[ch, u * 32:u * 32 + 32, :] = wbh[:, a::ANS].T
            wb[3 + ch, u * 32:u * 32 + 32, :] = wbl[:, a::ANS].T
            sv[u * 32:u * 32 + 32, ch] = _SCALES[a] / 256.0
    selr = np.zeros((4, 128, 128), np.float32)
    for r in range(4):
        for u in range(4):
            for c in range(32):
                selr[r, r * 32 + c, u * 32 + c] = 1.0
    wich, wicl = _f16_split(np.ascontiguousarray(W_ic.T, dtype=np.float32))
    wic = np.stack([wich, wicl])
    wach, wacl = _f16_split(np.ascontiguousarray(W_ac.T, dtype=np.float32))
    wac = np.stack([wach, wacl])

    in_maps = []
    for c in range(NCORES):
        rows = audio[c * BLOC:(c + 1) * BLOC]
        apad = np.zeros((BLOC, NPAD), np.float32)
        apad[:, PAD_L:PAD_L + N] = rows
        ah, al = _f16_split(apad)
        in_maps.append({"aph": ah, "apl": al, "lg": lg, "wb": wb,
                        "wic": wic, "wac": wac, "sv": sv, "selr": selr})
    return in_maps


def kernel(audio, gt_kernels, W_bushy, W_ic, W_ac, _trace=False):
    global _NC_CACHE
    if _NC_CACHE is None:
        _NC_CACHE = _build_nc()
    nc = _NC_CACHE
    in_maps = _prep_inputs(audio, gt_kernels, W_bushy, W_ic, W_ac)
    res = run_bass_kernel_spmd(nc, in_maps, core_ids=list(range(NCORES)),
                               trace=_trace)
    spk = np.empty((B, T, OUT), np.float32)
    mem = np.empty((B, T, OUT), np.float32)
    for c in range(NCORES):
        # [o, t*32+b] -> [b, t, o]
        spk[c * BLOC:(c + 1) * BLOC] = (
            res.results[c]["ospk"].reshape(OUT, T, BLOC).transpose(2, 1, 0))
        mem[c * BLOC:(c + 1) * BLOC] = (
            res.results[c]["omem"].reshape(OUT, T, BLOC).transpose(2, 1, 0))
    kernel._last_results = res
    return spk, mem


# revision 66
# speedup vs baseline: 1.0089x; 1.0089x over previous
"""Trainium2 Bass kernel for the gammatone-cochlea + LIF-SNN model (v2).

Numerically the SNN is chaotic: input perturbations of ~1e-5 relative flip
hundreds of output spikes (measured), so every value feeding a threshold
comparison must match the fp32 reference to <1e-6.  fp32 PE matmuls cost 4
cycles/col in the cost model; float32r costs 1 but its hardware numerics
(bf16 triple-product, ~3.4e-5) fail the gate.  Instead the conv runs as an
exact fp16 split: audio a = ah+al and kernel g = gh+gl (each an exact fp16
pair), and y = ah*gh + ah*gl + al*gh accumulated in fp32 PSUM (the dropped
al*gl term is ~2^-24, measured zero spike flips end-to-end).  Six 1-cyc/col
passes instead of two 4-cyc/col passes: 25% faster PE and exact.

Pipeline per core (32 of the 256 batch rows, pure data parallel), emitted
strip-major so the SNN wavefronts under the conv (PE ~92% busy end to end):
  for si in strips(2048 samples = 16 frames):
    for g in groups(4 rows): Hankel DMA (fp16 hi+lo) -> 6 matmul passes per
      512-block -> ReLU (Act, psum->sbuf) -> 128-block sums (DVE reduce)
    twice per strip (between conv groups, so the in-order PE queue never
    waits on LIF progress): env chunk (DVE) -> selector matmuls (PE, fp32)
      -> AN spikes (DVE, fp16 0/1) -> bushy currents (PE, fp16-split
      weights, exact on 0/1 spikes) -> Act copy to SBUF -> LIF chains
      (bushy chunk k, ic chunk k-1, ac chunk k-2, all on DVE: STT
      u=beta*mem+cur, is_gt spike, subtract reset; 3 ops/step/layer)
The last strip's chunks are split 7/4/3 frames and drained chunk-pipelined
after the conv using a 2-op negated-state form (u = -beta*m' + cur,
m' = (u>1) - u, bit-identical to the 3-op form) with spikes/outputs derived
in bulk per chunk, shortening the serial post-conv cascade; finished frames
stream out over a split output DMA.
Outputs [10, 124*32] (t-major) per core; host reassembles to [B, T, 10].
"""
import numpy as np
import concourse.bass as bass
import concourse.bacc as bacc
import concourse.mybir as mybir
import concourse.tile as tile
from concourse.bass_utils import run_bass_kernel_spmd

dt = mybir.dt
AF = mybir.ActivationFunctionType
OP = mybir.AluOpType

NCORES = 8
B, N, C, K = 256, 16000, 32, 64
BLOC = B // NCORES            # 32 batch rows per core
WINDOW, STRIDE, T = 256, 128, 124
ANS, HID, OUT = 10, 50, 10
BETA, THR, AN_THR = 0.95, 1.0, 0.5
PAD_L, PAD_R = 31, 33         # SAME padding for K=64: 31 left, 32 right (+1 slack)
NPAD = PAD_L + N + PAD_R      # 16064
FREE = T * BLOC               # 3968
NGRP = BLOC // 4              # 8 groups of 4 rows
STRIPS = [2048] * 7 + [1664]
NSTRIP = len(STRIPS)
S0S = [sum(STRIPS[:i]) for i in range(NSTRIP)]
# frame chunks: wave w = chunks ready once strip w's block sums exist.
# 7-8 frame chunks keep the ic/ac wavefront lag (and the post-conv drain)
# short.
CHUNKS = [(0, 7), (7, 14)] + \
    [c for i in range(1, 7)
     for c in ((16 * i - 2, 16 * i + 6), (16 * i + 6, 16 * i + 14))] + \
    [(110, 117), (117, 121), (121, 124)]

# jnp.linspace(0.5, 1.5, 10, dtype=f32), bitexact
_SCALES = np.array([0x3F000000, 0x3F1C71C7, 0x3F38E38E, 0x3F555555, 0x3F71C71D,
                    0x3F871C72, 0x3F955556, 0x3FA38E39, 0x3FB1C71D, 0x3FC00000],
                   dtype=np.uint32).view(np.float32)

_NC_CACHE = None


def _build_nc():
    nc = bacc.Bacc("TRN2", target_bir_lowering=False, debug=False,
                   num_devices=NCORES)

    aph = nc.dram_tensor("aph", [BLOC, NPAD], dt.float16, kind="ExternalInput")
    apl = nc.dram_tensor("apl", [BLOC, NPAD], dt.float16, kind="ExternalInput")
    # gammatone lhsT blocks: [hi/lo][tap-half] block-diagonal [128, 128]
    lg = nc.dram_tensor("lg", [128, 512], dt.float16, kind="ExternalInput")
    wb = nc.dram_tensor("wb", [6, 128, HID], dt.float16, kind="ExternalInput")
    wic = nc.dram_tensor("wic", [2, HID, HID], dt.float16, kind="ExternalInput")
    wac = nc.dram_tensor("wac", [2, HID, OUT], dt.float16, kind="ExternalInput")
    sv = nc.dram_tensor("sv", [128, 3], dt.float32, kind="ExternalInput")
    selr = nc.dram_tensor("selr", [4, 128, 128], dt.float32, kind="ExternalInput")
    ospk = nc.dram_tensor("ospk", [OUT, FREE], dt.float32, kind="ExternalOutput")
    omem = nc.dram_tensor("omem", [OUT, FREE], dt.float32, kind="ExternalOutput")

    with tile.TileContext(nc) as tc:
        with tc.tile_pool(name="cpool", bufs=1) as cp:
            lgt_all = cp.tile([128, 512], dt.float16)
            nc.scalar.dma_start(out=lgt_all[:, :], in_=lg[:, :])
            lgt = [lgt_all[:, 128 * i:128 * i + 128] for i in range(4)]
            svt = cp.tile([128, 3], dt.float32)
            nc.gpsimd.dma_start(out=svt[:, :], in_=sv[:, :])
            wbt = [cp.tile([128, HID], dt.float16, name=f"wbt{i}")
                   for i in range(6)]
            for i in range(6):
                nc.gpsimd.dma_start(out=wbt[i][:, :], in_=wb[i, :, :])
            wict = [cp.tile([HID, HID], dt.float16, name=f"wict{i}")
                    for i in range(2)]
            wact = [cp.tile([HID, OUT], dt.float16, name=f"wact{i}")
                    for i in range(2)]
            for i in range(2):
                nc.gpsimd.dma_start(out=wict[i][:, :], in_=wic[i, :, :])
                nc.gpsimd.dma_start(out=wact[i][:, :], in_=wac[i, :, :])
            selt = [cp.tile([128, 128], dt.float32, name=f"selt{r}")
                    for r in range(4)]
            for r in range(4):
                nc.gpsimd.dma_start(out=selt[r][:, :], in_=selr[r, :, :])

            S_all = cp.tile([128, NGRP * 126], dt.float32)
            env_all = cp.tile([128, NGRP * T], dt.float32)
            ospk_t = cp.tile([OUT, FREE], dt.float32,
                             padded_shape=[OUT, FREE + 32])
            omem_t = cp.tile([OUT, FREE], dt.float32,
                             padded_shape=[OUT, FREE + 32])

            # persistent LIF state
            memb = cp.tile([HID, BLOC], dt.float32)
            memic = cp.tile([HID, BLOC], dt.float32)
            zac = cp.tile([OUT, BLOC], dt.float32)
            nc.vector.memset(memb[:, :], 0.0)
            nc.vector.memset(memic[:, :], 0.0)
            nc.gpsimd.memset(zac[:, :], 0.0)

            hkp = tc.alloc_tile_pool(name="hkp", bufs=8)
            ysp = tc.alloc_tile_pool(name="ysp", bufs=4)
            anp = tc.alloc_tile_pool(name="anp", bufs=3)
            sbp = tc.alloc_tile_pool(name="sbp", bufs=1)
            psc = tc.alloc_tile_pool(name="psc", bufs=3, space="PSUM")
            pss = tc.alloc_tile_pool(name="pss", bufs=2, space="PSUM")
            psl = tc.alloc_tile_pool(name="psl", bufs=3, space="PSUM")

            def conv_strip(g, si):
                sw = STRIPS[si]
                s0 = S0S[si]
                hkh = hkp.tile([128, 2080], dt.float16, tag="hk", name="hkh")
                hkl = hkp.tile([128, 2080], dt.float16, tag="hk", name="hkl")
                # hk[32r+k, j] = ap[4g+r, s0+j+k] : one DMA, 128 partition rows
                if si == 0 and g == 0:
                    # split the very first transfers so the first conv matmul
                    # can start after only half a strip has landed
                    for (t, ap_) in ((hkh, aph), (hkl, apl)):
                        for (a, b) in ((0, 1056), (1056, sw + 32)):
                            src = bass.AP(ap_, 4 * g * NPAD + s0 + a,
                                          [[NPAD, 4], [1, 32], [1, b - a]])
                            nc.sync.dma_start(out=t[:, a:b], in_=src)
                else:
                    srch = bass.AP(aph, 4 * g * NPAD + s0,
                                   [[NPAD, 4], [1, 32], [1, sw + 32]])
                    srcl = bass.AP(apl, 4 * g * NPAD + s0,
                                   [[NPAD, 4], [1, 32], [1, sw + 32]])
                    nc.sync.dma_start(out=hkh[:, 0:sw + 32], in_=srch)
                    nc.sync.dma_start(out=hkl[:, 0:sw + 32], in_=srcl)
                ys = ysp.tile([128, 2048], dt.float32, tag="ys", name="ys")
                nb4 = (sw + 511) // 512
                for b4 in range(nb4):
                    w = min(512, sw - 512 * b4)
                    o = 512 * b4
                    acc = psc.tile([128, 512], dt.float32, tag="acc",
                                   name="acc")
                    # y = ah*gh + ah*gl + al*gh  (two tap-halves each)
                    nc.tensor.matmul(acc[:, 0:w], lgt[0],
                                     hkh[:, o:o + w], start=True, stop=False)
                    nc.tensor.matmul(acc[:, 0:w], lgt[1],
                                     hkh[:, o + 32:o + 32 + w],
                                     start=False, stop=False)
                    nc.tensor.matmul(acc[:, 0:w], lgt[2],
                                     hkh[:, o:o + w], start=False, stop=False)
                    nc.tensor.matmul(acc[:, 0:w], lgt[3],
                                     hkh[:, o + 32:o + 32 + w],
                                     start=False, stop=False)
                    nc.tensor.matmul(acc[:, 0:w], lgt[0],
                                     hkl[:, o:o + w], start=False, stop=False)
                    nc.tensor.matmul(acc[:, 0:w], lgt[1],
                                     hkl[:, o + 32:o + 32 + w],
                                     start=False, stop=True)
                    nc.scalar.activation(ys[:, o:o + w], acc[:, 0:w], AF.Relu)
                nblk = sw // 128
                b0 = s0 // 128
                if si == NSTRIP - 1:
                    # split the last strip's block sums: the first 8 blocks
                    # gate the drain's first LIF chunk, so finishing them
                    # before the tail blocks starts the drain sooner
                    for (ba, bb) in ((0, 8), (8, nblk)):
                        view = bass.AP(ys.tensor, ys.offset + 128 * ba,
                                       [list(ys.ap[0]), [128, bb - ba],
                                        [1, 128]])
                        nc.vector.tensor_reduce(
                            S_all[:, g * 126 + b0 + ba:
                                  g * 126 + b0 + bb],
                            view, axis=mybir.AxisListType.X, op=OP.add)
                else:
                    view = bass.AP(ys.tensor, ys.offset,
                                   [list(ys.ap[0]), [128, nblk], [1, 128]])
                    nc.vector.tensor_reduce(
                        S_all[:, g * 126 + b0: g * 126 + b0 + nblk],
                        view, axis=mybir.AxisListType.X, op=OP.add)

            def an_chunk(ci):
                t0, t1 = CHUNKS[ci]
                w = t1 - t0
                # env256[t] = S[t] + S[t+1]  (the /256 is folded into sv)
                dst = bass.AP(env_all.tensor, env_all.offset + t0,
                              [list(env_all.ap[0]), [T, NGRP], [1, w]])
                sA = bass.AP(S_all.tensor, S_all.offset + t0,
                             [list(S_all.ap[0]), [126, NGRP], [1, w]])
                sB = bass.AP(S_all.tensor, S_all.offset + t0 + 1,
                             [list(S_all.ap[0]), [126, NGRP], [1, w]])
                nc.vector.tensor_tensor(dst, sA, sB, OP.add)
                # replicate to (scale-block u, channel) partitions via 0/1
                # selector matmuls; chunk cols are b*w + dt, b = 4g+r
                shf = pss.tile([128, 32 * w], dt.float32, tag="shf",
                               name="shf")
                for r in range(4):
                    outap = bass.AP(shf.tensor, shf.offset + r * w,
                                    [list(shf.ap[0]), [4 * w, NGRP], [1, w]])
                    rhs = bass.AP(env_all.tensor, env_all.offset + t0,
                                  [list(env_all.ap[0]), [T, NGRP], [1, w]])
                    nc.tensor.matmul(outap, selt[r][:, :], rhs,
                                     start=True, stop=True)
                # AN spikes (3 passes cover the 10 scales) + bushy currents
                pb = psl.tile([HID, 32 * w], dt.float32, tag="pcur",
                              name="pb")
                for ch in range(3):
                    an = anp.tile([128, 32 * w], dt.float16, tag="an",
                                  name="an")
                    nc.vector.tensor_scalar(an[:, :], shf[:, :],
                                            svt[:, ch:ch + 1], AN_THR,
                                            OP.mult, OP.is_gt)
                    nc.tensor.matmul(pb[:, :], wbt[ch][:, :], an[:, :],
                                     start=(ch == 0), stop=False)
                    nc.tensor.matmul(pb[:, :], wbt[3 + ch][:, :], an[:, :],
                                     start=False, stop=(ch == 2))
                cb = sbp.tile([HID, 32 * w], dt.float32, tag="cb", bufs=2,
                              name="cb")
                nc.scalar.activation(cb[:, :], pb[:, :], AF.Copy)
                return cb

            def ic_cur(ci, sb):
                t0, t1 = CHUNKS[ci]
                w = t1 - t0
                pi = psl.tile([HID, 32 * w], dt.float32, tag="pcur",
                              name="pi")
                nc.tensor.matmul(pi[:, :], wict[0][:, :], sb[:, :],
                                 start=True, stop=False)
                nc.tensor.matmul(pi[:, :], wict[1][:, :], sb[:, :],
                                 start=False, stop=True)
                cic = sbp.tile([HID, 32 * w], dt.float32, tag="cic", bufs=2,
                               name="cic")
                nc.scalar.activation(cic[:, :], pi[:, :], AF.Copy)
                return cic

            def ac_cur(ci, sic):
                t0, t1 = CHUNKS[ci]
                w = t1 - t0
                pa = psl.tile([OUT, 32 * w], dt.float32, tag="pcur",
                              name="pa")
                nc.tensor.matmul(pa[:, :], wact[0][:, :], sic[:, :],
                                 start=True, stop=False)
                nc.tensor.matmul(pa[:, :], wact[1][:, :], sic[:, :],
                                 start=False, stop=True)
                cac = sbp.tile([OUT, 32 * w], dt.float32, tag="cac", bufs=2,
                               name="cac")
                nc.scalar.activation(cac[:, :], pa[:, :], AF.Copy)
                return cac

            def csl(tile2d, w, dtc):
                """column slice {b*w + dtc} of a chunk tile."""
                return bass.AP(tile2d.tensor, tile2d.offset + dtc,
                               [list(tile2d.ap[0]), [w, BLOC]])

            def bushy_steps(ci, cb, ub, sbch):
                t0, t1 = CHUNKS[ci]
                w = t1 - t0
                for dtc in range(w):
                    nc.vector.scalar_tensor_tensor(
                        ub[:, :], memb[:, :], BETA, csl(cb, w, dtc),
                        OP.mult, OP.add)
                    yield
                    nc.vector.tensor_scalar(csl(sbch, w, dtc), ub[:, :],
                                            THR, None, OP.is_gt)
                    yield
                    nc.vector.tensor_tensor(memb[:, :], ub[:, :],
                                            csl(sbch, w, dtc), OP.subtract)
                    yield

            def ic_steps(ci, cic, uic, sicch):
                t0, t1 = CHUNKS[ci]
                w = t1 - t0
                for dtc in range(w):
                    nc.vector.scalar_tensor_tensor(
                        uic[:, :], memic[:, :], BETA, csl(cic, w, dtc),
                        OP.mult, OP.add)
                    yield
                    nc.vector.tensor_scalar(csl(sicch, w, dtc), uic[:, :],
                                            THR, None, OP.is_gt)
                    yield
                    nc.vector.tensor_tensor(memic[:, :], uic[:, :],
                                            csl(sicch, w, dtc), OP.subtract)
                    yield

            def ac_steps(ci, cac, uac):
                # ospk_t/omem_t are t-major: frame t = cols [32t, 32t+32)
                t0, t1 = CHUNKS[ci]
                w = t1 - t0
                for dtc in range(w):
                    t = t0 + dtc
                    prev = (zac[:, :] if t == 0
                            else omem_t[:, 32 * t - 32:32 * t])
                    nc.vector.scalar_tensor_tensor(
                        uac[:, :], prev, BETA, csl(cac, w, dtc),
                        OP.mult, OP.add)
                    yield
                    nc.vector.tensor_scalar(ospk_t[:, 32 * t:32 * t + 32],
                                            uac[:, :], THR, None, OP.is_gt)
                    yield
                    nc.vector.tensor_tensor(omem_t[:, 32 * t:32 * t + 32],
                                            uac[:, :],
                                            ospk_t[:, 32 * t:32 * t + 32],
                                            OP.subtract)
                    yield

            def run_chains(chains):
                its = [c for c in chains if c is not None]
                done = [False] * len(its)
                while not all(done):
                    for k, it in enumerate(its):
                        if not done[k]:
                            try:
                                next(it)
                            except StopIteration:
                                done[k] = True

            # ---------------- main wavefront ----------------
            sb_ch = {}      # bushy spike chunks (fp16)
            sic_ch = {}     # ic spike chunks

            def bushy_for(ci):
                cb = an_chunk(ci)
                wd = CHUNKS[ci][1] - CHUNKS[ci][0]
                ub = sbp.tile([HID, BLOC], dt.float32, tag="ub",
                              bufs=2, name="ub")
                sbch = sbp.tile([HID, 32 * wd], dt.float16,
                                tag="sbch", bufs=4, name="sbch")
                sb_ch[ci] = sbch
                yield from bushy_steps(ci, cb, ub, sbch)

            def ic_for(ci):
                cic = ic_cur(ci, sb_ch.pop(ci))
                wd = CHUNKS[ci][1] - CHUNKS[ci][0]
                uic = sbp.tile([HID, BLOC], dt.float32,
                               tag="uic", bufs=2, name="uic")
                sicch = sbp.tile([HID, 32 * wd], dt.float16,
                                 tag="sicch", bufs=4, name="sicch")
                sic_ch[ci] = sicch
                yield from ic_steps(ci, cic, uic, sicch)

            def ac_for(ci):
                cac = ac_cur(ci, sic_ch.pop(ci))
                uac = sbp.tile([OUT, BLOC], dt.float32,
                               tag="uac", bufs=2, name="uac")
                yield from ac_steps(ci, cac, uac)

            def drain_layer(cis, P, get_cur, on_chunk, state0, tg, ac_u=None):
                """Drain-only 2-op LIF: u = (state*±beta)+cur, m' = (u>1)-u.
                The state is the previous m' column (negated mem) except the
                very first step, which reads the in-loop positive state with
                +beta (exact: -beta*m' == beta*mem bit-for-bit).  For the ac
                layer, u goes t-major into ac_u so spk/mem derive in bulk."""
                prev, pb = state0, BETA
                for ci in cis:
                    t0, t1 = CHUNKS[ci]
                    w = t1 - t0
                    if ac_u is not None:
                        # wait until the ic drain generator has emitted this
                        # chunk's spike derivation before consuming it
                        while ci not in sic_ch:
                            yield
                    cur = get_cur(ci)
                    mch = sbp.tile([P, 32 * w], dt.float32, tag="dm" + tg,
                                   bufs=3, name="m2" + tg)
                    if ac_u is None:
                        uch = sbp.tile([P, 32 * w], dt.float32, tag="du" + tg,
                                       bufs=3, name="u2" + tg)
                        def ucol(d, _u=uch, _w=w):
                            return csl(_u, _w, d)
                    else:
                        def ucol(d, _t0=t0):
                            a = 32 * (_t0 - 94 + d)
                            return ac_u[:, a:a + 32]
                    for dtc in range(w):
                        nc.vector.scalar_tensor_tensor(
                            ucol(dtc), prev, pb, csl(cur, w, dtc),
                            OP.mult, OP.add)
                        yield
                        u = ucol(dtc)
                        nc.vector.scalar_tensor_tensor(
                            csl(mch, w, dtc), u, THR, u,
                            OP.is_gt, OP.subtract)
                        yield
                        prev, pb = csl(mch, w, dtc), -BETA
                    if ac_u is None:
                        on_chunk(ci, uch, w)

            def drain_bushy_done(ci, uch, w):
                sbch = sbp.tile([HID, 32 * w], dt.float16, tag="sbch",
                                bufs=4, name="sbch")
                nc.vector.tensor_scalar(sbch[:, :], uch[:, :], THR, None,
                                        OP.is_gt)
                sb_ch[ci] = sbch

            def drain_ic_done(ci, uch, w):
                sicch = sbp.tile([HID, 32 * w], dt.float16, tag="sicch",
                                 bufs=4, name="sicch")
                nc.vector.tensor_scalar(sicch[:, :], uch[:, :], THR, None,
                                        OP.is_gt)
                sic_ch[ci] = sicch

            def seq(factory, cis):
                for ci in cis:
                    yield from factory(ci)

            def chunk_work(b_ci, ic_ci, ac_ci):
                """One chunk per chain, staggered: every matmul depends only
                on work finished before this point, so the in-order PE queue
                never stalls on LIF progress."""
                nch = len(CHUNKS)
                chains = []
                if b_ci is not None and 0 <= b_ci < nch:
                    chains.append(bushy_for(b_ci))
                if ic_ci is not None and 0 <= ic_ci < nch:
                    chains.append(ic_for(ic_ci))
                if ac_ci is not None and 0 <= ac_ci < nch:
                    chains.append(ac_for(ac_ci))
                run_chains(chains)

            # PE warmup: ~3us of tiny serialized matmuls while the first
            # Hankel DMA lands, so the real conv starts at full clock (the
            # cost model ramps the PE to peak after 3us of continuous busy)
            wz = cp.tile([128, 64], dt.float16)
            nc.vector.memset(wz[:, :], 0.0)
            pw = psc.tile([128, 512], dt.float32, tag="acc", name="warm")
            for _ in range(34):
                nc.tensor.matmul(pw[0:64, 0:64], wz[:, :], wz[:, :],
                                 start=True, stop=True)

            for si in range(NSTRIP):
                conv_strip(0, si)
                if si >= 1:
                    # chunk work for the previous strip, emitted between
                    # conv groups so the env/spike chains are ready by the
                    # time the PE reaches their matmuls
                    chunk_work(2 * si - 2, 2 * si - 3, 2 * si - 4)
                for g in range(1, 4):
                    conv_strip(g, si)
                if si >= 1:
                    chunk_work(2 * si - 1, 2 * si - 2, 2 * si - 3)
                for g in range(4, 6):
                    conv_strip(g, si)
                if si == NSTRIP - 1:
                    # last strip only: pull ac chunk 12 into the loop (its
                    # ic spikes completed at point B above), shortening the
                    # post-conv ac cascade by 8 frames
                    chunk_work(None, None, 12)
                for g in range(6, NGRP):
                    conv_strip(g, si)
            # frames < 94 are final once ac chunk 11 is done (in-loop);
            # ship them while the drain runs
            CUT = 32 * 102
            nc.sync.dma_start(out=ospk[:, 0:CUT], in_=ospk_t[:, 0:CUT])
            nc.sync.dma_start(out=omem[:, 0:CUT], in_=omem_t[:, 0:CUT])
            # post-conv drain: chunk-pipelined 2-op chains; the ac membrane
            # u is collected t-major in uacd and spk/mem derived in bulk
            uacd = cp.tile([OUT, 32 * 30], dt.float32)
            run_chains([
                drain_layer([14, 15, 16], HID,
                            lambda ci: an_chunk(ci), drain_bushy_done,
                            memb[:, :], "b"),
                drain_layer([13, 14, 15, 16], HID,
                            lambda ci: ic_cur(ci, sb_ch.pop(ci)),
                            drain_ic_done, memic[:, :], "i"),
                drain_layer([13, 14, 15, 16], OUT,
                            lambda ci: ac_cur(ci, sic_ch.pop(ci)),
                            None, omem_t[:, 32 * 101:32 * 102], "a",
                            ac_u=uacd),
            ])
            dsl = slice(32 * 102, FREE)
            ua = uacd[:, 32 * 8:32 * 30]
            nc.vector.tensor_scalar(ospk_t[:, dsl], ua, THR, None,
                                    OP.is_gt)
            nc.vector.tensor_tensor(omem_t[:, dsl], ua,
                                    ospk_t[:, dsl], OP.subtract)
            nc.sync.dma_start(out=ospk[:, CUT:FREE], in_=ospk_t[:, CUT:FREE])
            nc.sync.dma_start(out=omem[:, CUT:FREE], in_=omem_t[:, CUT:FREE])

            nc.sync.dma_start(out=ospk[:, CUT:FREE], in_=ospk_t[:, CUT:FREE])
            nc.sync.dma_start(out=omem[:, CUT:FREE], in_=omem_t[:, CUT:FREE])

            psl.release()
            pss.release()
            psc.release()
            sbp.release()
            anp.release()
            ysp.release()
            hkp.release()

    nc.finalize()
    return nc


def _f16_split(x):
    hi = x.astype(np.float16)
    lo = (x.astype(np.float32) - hi.astype(np.float32)).astype(np.float16)
    return hi, lo


def _prep_inputs(audio, gt_kernels, W_bushy, W_ic, W_ac):
    audio = np.ascontiguousarray(audio, dtype=np.float32)
    gt = np.ascontiguousarray(gt_kernels, dtype=np.float32)
    Wb = np.ascontiguousarray(W_bushy, dtype=np.float32)

    gh, gl = _f16_split(gt)
    lg4 = np.zeros((4, 128, 128), np.float16)
    for r in range(4):
        # lhsT[r*32+k, r*32+c] = g[c, k]; [0]=gh taps0-31, [1]=gh taps32-63,
        # [2]=gl taps0-31, [3]=gl taps32-63
        lg4[0, r * 32:r * 32 + 32, r * 32:r * 32 + 32] = gh[:, 0:32].T
        lg4[1, r * 32:r * 32 + 32, r * 32:r * 32 + 32] = gh[:, 32:64].T
        lg4[2, r * 32:r * 32 + 32, r * 32:r * 32 + 32] = gl[:, 0:32].T
        lg4[3, r * 32:r * 32 + 32, r * 32:r * 32 + 32] = gl[:, 32:64].T
    lg = np.ascontiguousarray(np.concatenate(list(lg4), axis=1))

    wbh, wbl = _f16_split(Wb)       # [50, 320]
    wb = np.zeros((6, 128, HID), np.float16)
    sv = np.zeros((128, 3), np.float32)
    for ch in range(3):
        for u in range(4):
            a = ch * 4 + u
            if a >= ANS:
                continue
            wb[ch, u * 32:u * 32 + 32, :] = wbh[:, a::ANS].T
            wb[3 + ch, u * 32:u * 32 + 32, :] = wbl[:, a::ANS].T
            sv[u * 32:u * 32 + 32, ch] = _SCALES[a] / 256.0
    selr = np.zeros((4, 128, 128), np.float32)
    for r in range(4):
        for u in range(4):
            for c in range(32):
                selr[r, r * 32 + c, u * 32 + c] = 1.0
    wich, wicl = _f16_split(np.ascontiguousarray(W_ic.T, dtype=np.float32))
    wic = np.stack([wich, wicl])
    wach, wacl = _f16_split(np.ascontiguousarray(W_ac.T, dtype=np.float32))
    wac = np.stack([wach, wacl])

    in_maps = []
    for c in range(NCORES):
        rows = audio[c * BLOC:(c + 1) * BLOC]
        apad = np.zeros((BLOC, NPAD), np.float32)
        apad[:, PAD_L:PAD_L + N] = rows
        ah, al = _f16_split(apad)
        in_maps.append({"aph": ah, "apl": al, "lg": lg, "wb": wb,
                        "wic": wic, "wac": wac, "sv": sv, "selr": selr})
    return in_maps


def kernel(audio, gt_kernels, W_bushy, W_ic, W_ac, _trace=False):
    global _NC_CACHE
    if _NC_CACHE is None:
        _NC_CACHE = _build_nc()
    nc = _NC_CACHE
    in_maps = _prep_inputs(audio, gt_kernels, W_bushy, W_ic, W_ac)
    res = run_bass_kernel_spmd(nc, in_maps, core_ids=list(range(NCORES)),
                               trace=_trace)
    spk = np.empty((B, T, OUT), np.float32)
    mem = np.empty((B, T, OUT), np.float32)
    for c in range(NCORES):
        # [o, t*32+b] -> [b, t, o]
        spk[c * BLOC:(c + 1) * BLOC] = (
            res.results[c]["ospk"].reshape(OUT, T, BLOC).transpose(2, 1, 0))
        mem[c * BLOC:(c + 1) * BLOC] = (
            res.results[c]["omem"].reshape(OUT, T, BLOC).transpose(2, 1, 0))
    kernel._last_results = res
    return spk, mem
